# revision 2
# baseline (speedup 1.0000x reference)
"""Trainium2 Bass kernel for a single-step 4-layer LSTM decoder with
Bahdanau attention and a 50k-vocab output projection, SPMD across 8
NeuronCores.

Distribution strategy (hardcoded for B=32, S=2048, H=E=1024, V=50257, L=4):
  - Attention is data-parallel over batch: 4 batches per core, streaming
    encoder_outputs once (online unnormalized softmax; the additive-attention
    hidden term v.(Wa_h@out+ba) is constant per row and cancels in softmax,
    and v folds through Wa_e on the host into a single vector u, so the
    whole attention scan is independent of the LSTM).
  - LSTM is tensor-parallel over hidden units: 128 units per core across all
    4 gates, with a small AllGather of h after each layer.
  - Output layer is tensor-parallel over vocab: Wout is pre-transposed,
    zero-padded to 51200 and column-sharded 6400 per core.
  - The embedding table is never shipped: the two gathers (64 rows) happen
    on the host and feed the kernel as a 2048x32 transposed input.

All weight matrices are pre-transposed on the host so every DMA is
contiguous along the partition line.
"""

import os

import numpy as np

B, S, H, E, V, L = 32, 2048, 1024, 1024, 50257, 4
NC = 8
BPC = B // NC          # batches per core (attention)
UPC = H // NC          # hidden units per core (LSTM)
VPAD = 51200           # vocab padded to 8*6400
VS = VPAD // NC        # vocab shard per core
ST = S // 128          # s-tiles per batch (16)
LGC = 512              # logits column chunk

_COMPILED = {}
LAST_EXEC_TIME_NS = None


def _build_nc():
    import concourse.bacc as bacc
    import concourse.mybir as mybir
    import concourse.tile as tile

    f32 = mybir.dt.float32
    AF = mybir.ActivationFunctionType
    OP = mybir.AluOpType

    nc = bacc.Bacc("TRN2", target_bir_lowering=False, debug=False, num_devices=NC)

    # ---- parameters ----
    encs = nc.declare_dram_parameter("encs", [BPC, S, H], f32, isOutput=False)
    xT = nc.declare_dram_parameter("xT", [2 * E, B], f32, isOutput=False)
    h0T = nc.declare_dram_parameter("h0T", [L, H, B], f32, isOutput=False)
    c0T = nc.declare_dram_parameter("c0T", [L, UPC, B], f32, isOutput=False)
    u_p = nc.declare_dram_parameter("u", [1, H], f32, isOutput=False)
    wih0T = nc.declare_dram_parameter("wih0T", [2 * E, 4 * UPC], f32, isOutput=False)
    whh0T = nc.declare_dram_parameter("whh0T", [H, 4 * UPC], f32, isOutput=False)
    wihrT = nc.declare_dram_parameter("wihrT", [L - 1, H, 4 * UPC], f32, isOutput=False)
    whhrT = nc.declare_dram_parameter("whhrT", [L - 1, H, 4 * UPC], f32, isOutput=False)
    b0_p = nc.declare_dram_parameter("b0", [UPC, 4], f32, isOutput=False)
    br_p = nc.declare_dram_parameter("br", [UPC, 4 * (L - 1)], f32, isOutput=False)
    walT = nc.declare_dram_parameter("walT", [2 * H + E, UPC], f32, isOutput=False)
    bal_p = nc.declare_dram_parameter("bal", [UPC, 1], f32, isOutput=False)
    woutT = nc.declare_dram_parameter("woutT", [H, VS], f32, isOutput=False)
    bout_p = nc.declare_dram_parameter("bout", [1, VS], f32, isOutput=False)

    logits_o = nc.declare_dram_parameter("logits_o", [B, VS], f32, isOutput=True)
    hnT_o = nc.declare_dram_parameter("hnT", [L, UPC, B], f32, isOutput=True)
    cnT_o = nc.declare_dram_parameter("cnT", [L, UPC, B], f32, isOutput=True)
    attnsT_o = nc.declare_dram_parameter("attnsT", [BPC, S], f32, isOutput=True)

    grp = [list(range(NC))]

    with tile.TileContext(nc) as tc:
        with (
            tc.tile_pool(name="sb", bufs=1) as sb,
            tc.tile_pool(name="ps", bufs=1, space="PSUM") as ps,
            tc.tile_pool(name="dram", bufs=1, space="DRAM") as dram,
        ):
            # ---------- prelude: small constants ----------
            ones = sb.tile([128, 32], f32, name="ones")
            nc.vector.memset(ones[:], 1.0)
            u_bc = sb.tile([128, H], f32, name="u_bc")
            nc.sync.dma_start(u_bc[:], u_p.ap().broadcast_to([128, H]))
            b0_sb = sb.tile([UPC, 4], f32, name="b0_sb")
            nc.sync.dma_start(b0_sb[:], b0_p[:])
            br_sb = sb.tile([UPC, 4 * (L - 1)], f32, name="br_sb")
            nc.sync.dma_start(br_sb[:], br_p[:])
            bal_sb = sb.tile([UPC, 1], f32, name="bal_sb")
            nc.sync.dma_start(bal_sb[:], bal_p[:])
            c0_sb = []
            for l in range(L):
                c0_l = sb.tile([UPC, B], f32, name=f"c0_{l}")
                nc.sync.dma_start(c0_l[:], c0T[l])
                c0_sb.append(c0_l)
            bout_sb = sb.tile([1, VS], f32, name="bout_sb")
            nc.sync.dma_start(bout_sb[:], bout_p[:])

            # DRAM bounce buffers for collectives
            ag_in = [dram.tile([UPC, B], f32, name=f"ag_in{l}") for l in range(L)]
            ag_out = [dram.tile([H, B], f32, name=f"ag_out{l}") for l in range(L)]
            ctxT_loc = dram.tile([H, BPC], f32, name="ctxT_loc")
            ctx_g = dram.tile([NC * H, BPC], f32, name="ctx_g")
            hid_in = dram.tile([UPC, B], f32, name="hid_in")
            hid_out = dram.tile([H, B], f32, name="hid_out")
            invd_dram = dram.tile([1, BPC], f32, name="invd_dram")

            # ---------- LSTM: 4 layers, hidden-sharded, AllGather per layer ----------
            gp = ps.tile([128, 128], f32, name="gp", bufs=2)

            for l in range(L):
                if l == 0:
                    n_x = (2 * E) // 128  # 16
                    wsrc = [(wih0T, j) for j in range(n_x)] + [
                        (whh0T, j) for j in range(H // 128)
                    ]
                    rsrc = [(xT, j) for j in range(n_x)] + [
                        (h0T, (0, j)) for j in range(H // 128)
                    ]
                else:
                    wsrc = [(wihrT, (l - 1, j)) for j in range(H // 128)] + [
                        (whhrT, (l - 1, j)) for j in range(H // 128)
                    ]
                    rsrc = [(ag_out[l - 1], j) for j in range(H // 128)] + [
                        (h0T, (l, j)) for j in range(H // 128)
                    ]
                kt = len(wsrc)

                w_tiles = []
                for k, (src, idx) in enumerate(wsrc):
                    wt = sb.tile([128, 4 * UPC], f32, name="lw", tag="lw", bufs=26)
                    if isinstance(idx, tuple):
                        li, j = idx
                        nc.sync.dma_start(wt[:], src[li, j * 128:(j + 1) * 128, :])
                    else:
                        nc.sync.dma_start(wt[:], src[idx * 128:(idx + 1) * 128, :])
                    w_tiles.append(wt)
                r_tiles = []
                for k, (src, idx) in enumerate(rsrc):
                    rt = sb.tile([128, B], f32, name="rk", tag="rk", bufs=26)
                    if isinstance(idx, tuple):
                        li, j = idx
                        nc.gpsimd.dma_start(rt[:], src[li, j * 128:(j + 1) * 128, :])
                    else:
                        nc.gpsimd.dma_start(rt[:], src[idx * 128:(idx + 1) * 128, :])
                    r_tiles.append(rt)

                gpt = gp if l == 0 else ps.tile([128, 128], f32, name="gp", tag="gp", bufs=2)
                for gi in range(4):
                    for k in range(kt):
                        nc.tensor.matmul(
                            gpt[:, gi * 32:(gi + 1) * 32],
                            w_tiles[k][:, gi * UPC:(gi + 1) * UPC],
                            r_tiles[k][:],
                            start=(k == 0),
                            stop=(k == kt - 1),
                        )

                bias = b0_sb if l == 0 else br_sb
                boff = 0 if l == 0 else 4 * (l - 1)
                s_i = sb.tile([UPC, B], f32, name="s_i", tag="s_i", bufs=2)
                s_f = sb.tile([UPC, B], f32, name="s_f", tag="s_f", bufs=2)
                tg = sb.tile([UPC, B], f32, name="tg", tag="tg", bufs=2)
                s_o = sb.tile([UPC, B], f32, name="s_o", tag="s_o", bufs=2)
                nc.scalar.activation(s_i[:], gpt[:, 0:32], AF.Sigmoid, bias=bias[:, boff + 0:boff + 1])
                nc.scalar.activation(s_f[:], gpt[:, 32:64], AF.Sigmoid, bias=bias[:, boff + 1:boff + 2])
                nc.scalar.activation(tg[:], gpt[:, 64:96], AF.Tanh, bias=bias[:, boff + 2:boff + 3])
                nc.scalar.activation(s_o[:], gpt[:, 96:128], AF.Sigmoid, bias=bias[:, boff + 3:boff + 4])

                c_new = sb.tile([UPC, B], f32, name="c_new", tag="c_new", bufs=2)
                h_new = sb.tile([UPC, B], f32, name="h_new", tag="h_new", bufs=2)
                tcn = sb.tile([UPC, B], f32, name="tcn", tag="tcn", bufs=2)
                m2 = sb.tile([UPC, B], f32, name="m2", tag="m2", bufs=2)
                nc.vector.tensor_mul(c_new[:], s_f[:], c0_sb[l][:])
                nc.vector.tensor_mul(m2[:], s_i[:], tg[:])
                nc.vector.tensor_add(c_new[:], c_new[:], m2[:])
                nc.scalar.activation(tcn[:], c_new[:], AF.Tanh)
                nc.vector.tensor_mul(h_new[:], s_o[:], tcn[:])

                nc.gpsimd.dma_start(cnT_o[l], c_new[:])
                nc.gpsimd.dma_start(hnT_o[l], h_new[:])
                nc.gpsimd.dma_start(ag_in[l][:], h_new[:])
                nc.gpsimd.collective_compute(
                    "AllGather", OP.bypass, replica_groups=grp,
                    ins=[ag_in[l].opt()], outs=[ag_out[l].opt()],
                )

            # ---------- attention: 4 batches/core, stream enc once ----------
            e_all, p_all, acc_b = [], [], []
            for b in range(BPC):
                e_all.append(sb.tile([128, ST], f32, name=f"e_all{b}"))
                p_all.append(sb.tile([128, ST], f32, name=f"p_all{b}"))
                acc_b.append(sb.tile([128, H], f32, name=f"acc{b}"))
            scratch = sb.tile([128, H], f32, name="scratch")
            d_ps = ps.tile([1, BPC], f32, name="d_ps")
            ctxf = ps.tile([128, 8], f32, name="ctxf", bufs=2)

            for b in range(BPC):
                for t in range(ST):
                    et = sb.tile([128, H], f32, name="enc_t", tag="enc_t", bufs=6)
                    nc.sync.dma_start(et[:], encs[b, t * 128:(t + 1) * 128, :])
                    nc.vector.scalar_tensor_tensor(
                        out=scratch[:], in0=et[:], scalar=1.0, in1=u_bc[:],
                        op0=OP.mult, op1=OP.mult, accum_out=e_all[b][:, t:t + 1],
                    )
                    nc.scalar.activation(
                        p_all[b][:, t:t + 1], e_all[b][:, t:t + 1], AF.Exp
                    )
                    if t == 0:
                        nc.vector.tensor_scalar_mul(
                            acc_b[b][:], et[:], p_all[b][:, 0:1]
                        )
                    else:
                        nc.vector.scalar_tensor_tensor(
                            out=acc_b[b][:], in0=et[:], scalar=p_all[b][:, t:t + 1],
                            in1=acc_b[b][:], op0=OP.mult, op1=OP.add,
                        )
                d_col = sb.tile([128, 1], f32, name="d_col", tag="d_col", bufs=2)
                nc.vector.reduce_sum(d_col[:], p_all[b][:], axis=mybir.AxisListType.X)
                nc.tensor.matmul(
                    d_ps[0:1, b:b + 1], d_col[:], ones[:, 0:1], start=True, stop=True
                )

            d_sb = sb.tile([1, BPC], f32, name="d_sb")
            inv_d = sb.tile([1, BPC], f32, name="inv_d")
            nc.vector.tensor_copy(d_sb[:], d_ps[:])
            nc.vector.reciprocal(inv_d[:], d_sb[:])
            nc.gpsimd.dma_start(invd_dram[:], inv_d[:])
            invd_bc = sb.tile([128, BPC], f32, name="invd_bc")
            nc.gpsimd.dma_start(invd_bc[:], invd_dram.broadcast_to([128, BPC]))

            for b in range(BPC):
                at_sb = sb.tile([128, ST], f32, name="at_sb", tag="at_sb", bufs=2)
                nc.vector.tensor_scalar_mul(at_sb[:], p_all[b][:], invd_bc[:, b:b + 1])
                nc.gpsimd.dma_start(
                    attnsT_o[b].rearrange("(t p) -> p t", p=128), at_sb[:]
                )
                ctxft = ctxf if b == 0 else ps.tile([128, 8], f32, name="ctxf", tag="ctxf", bufs=2)
                for m in range(8):
                    nc.tensor.matmul(
                        ctxft[:, m:m + 1],
                        acc_b[b][:, m * 128:(m + 1) * 128],
                        ones[:, 0:1],
                        start=True, stop=True,
                    )
                cx_sb = sb.tile([128, 8], f32, name="cx_sb", tag="cx_sb", bufs=2)
                nc.vector.tensor_scalar_mul(cx_sb[:], ctxft[:], invd_bc[:, b:b + 1])
                nc.gpsimd.dma_start(
                    ctxT_loc[:, b].rearrange("(m p) -> p m", p=128), cx_sb[:]
                )

            nc.gpsimd.collective_compute(
                "AllGather", OP.bypass, replica_groups=grp,
                ins=[ctxT_loc.opt()], outs=[ctx_g.opt()],
            )

            # ---------- hidden = tanh(Wal @ [out; context; key_e] + bal), sharded ----------
            wal_tiles = []
            KT = (2 * H + E) // 128  # 24
            for k in range(KT):
                wt = sb.tile([128, UPC], f32, name="walt", tag="walt", bufs=KT)
                nc.sync.dma_start(wt[:], walT[k * 128:(k + 1) * 128, :])
                wal_tiles.append(wt)
            hr_tiles = []
            ctx_g3 = ctx_g.rearrange("(c h) b -> h c b", c=NC)
            for k in range(KT):
                rt = sb.tile([128, B], f32, name="hrk", tag="rk", bufs=26)
                if k < 8:
                    nc.gpsimd.dma_start(rt[:], ag_out[L - 1][k * 128:(k + 1) * 128, :])
                elif k < 16:
                    j = k - 8
                    nc.gpsimd.dma_start(
                        rt.rearrange("p (c b) -> p c b", c=NC),
                        ctx_g3[j * 128:(j + 1) * 128],
                    )
                else:
                    j = k - 16
                    nc.gpsimd.dma_start(rt[:], xT[E + j * 128:E + (j + 1) * 128, :])
                hr_tiles.append(rt)

            hid_ps = ps.tile([UPC, B], f32, name="hid_ps")
            for k in range(KT):
                nc.tensor.matmul(
                    hid_ps[:], wal_tiles[k][:], hr_tiles[k][:],
                    start=(k == 0), stop=(k == KT - 1),
                )
            hidT_sb = sb.tile([UPC, B], f32, name="hidT_sb")
            nc.scalar.activation(hidT_sb[:], hid_ps[:], AF.Tanh, bias=bal_sb[:, 0:1])
            nc.gpsimd.dma_start(hid_in[:], hidT_sb[:])
            nc.gpsimd.collective_compute(
                "AllGather", OP.bypass, replica_groups=grp,
                ins=[hid_in.opt()], outs=[hid_out.opt()],
            )
            hT_sb = sb.tile([128, 8 * B], f32, name="hT_sb")
            for k in range(8):
                nc.gpsimd.dma_start(
                    hT_sb[:, k * B:(k + 1) * B], hid_out[k * 128:(k + 1) * 128, :]
                )

            # ---------- logits: vocab-sharded output projection ----------
            n_ch = (VS + LGC - 1) // LGC
            for ch in range(n_ch):
                c0_ = ch * LGC
                cw = min(LGC, VS - c0_)
                lg = ps.tile([B, LGC], f32, name="lg", tag="lg", bufs=2)
                for k in range(8):
                    wt = sb.tile([128, LGC], f32, name="wo", tag="wo", bufs=4)
                    nc.sync.dma_start(
                        wt[:, 0:cw], woutT[k * 128:(k + 1) * 128, c0_:c0_ + cw]
                    )
                    nc.tensor.matmul(
                        lg[:, 0:cw], hT_sb[:, k * B:(k + 1) * B], wt[:, 0:cw],
                        start=(k == 0), stop=False,
                    )
                nc.tensor.matmul(
                    lg[:, 0:cw], ones[0:1, :], bout_sb[0:1, c0_:c0_ + cw],
                    start=False, stop=True,
                )
                lsb = sb.tile([B, LGC], f32, name="lsb", tag="lsb", bufs=3)
                nc.vector.tensor_copy(lsb[:, 0:cw], lg[:, 0:cw])
                nc.sync.dma_start(logits_o[:, c0_:c0_ + cw], lsb[:, 0:cw])

    nc.finalize()
    return nc


def _host_prep(encoder_outputs, input_seq, keyword, h0, c0, emb,
               Wih0, Whh0, bih0, bhh0, Wih_r, Whh_r, bih_r, bhh_r,
               Wa, ba, v, Wal, bal, Wout, bout):
    f = np.float32
    enc = np.asarray(encoder_outputs, f)
    emb = np.asarray(emb, f)
    idx_sos = np.asarray(input_seq).astype(np.int64)[:, 0]
    idx_key = np.asarray(keyword).astype(np.int64)
    sos = emb[idx_sos]                      # [B, E]
    key_e = emb[idx_key]                    # [B, E]
    xT = np.ascontiguousarray(np.concatenate([sos, key_e], axis=1).T)  # [2E, B]

    Wa = np.asarray(Wa, f)
    v_ = np.asarray(v, f)
    u = (Wa[:, H:].T @ v_)[None, :].astype(f)  # [1, H]

    h0 = np.asarray(h0, f)
    c0 = np.asarray(c0, f)
    h0T = np.ascontiguousarray(h0.transpose(0, 2, 1))  # [L, H, B]

    Wih0 = np.asarray(Wih0, f).reshape(4, H, 2 * E)
    Whh0 = np.asarray(Whh0, f).reshape(4, H, H)
    Wih_r = np.asarray(Wih_r, f).reshape(L - 1, 4, H, H)
    Whh_r = np.asarray(Whh_r, f).reshape(L - 1, 4, H, H)
    b0_full = (np.asarray(bih0, f) + np.asarray(bhh0, f)).reshape(4, H)
    br_full = (np.asarray(bih_r, f) + np.asarray(bhh_r, f)).reshape(L - 1, 4, H)

    Wal = np.asarray(Wal, f)
    bal = np.asarray(bal, f)
    Wout = np.asarray(Wout, f)
    bout = np.asarray(bout, f)

    woutT_pad = np.zeros((H, VPAD), f)
    woutT_pad[:, :V] = Wout.T
    bout_pad = np.zeros(VPAD, f)
    bout_pad[:V] = bout

    in_maps = []
    for c in range(NC):
        sl = slice(c * UPC, (c + 1) * UPC)
        wih0T = np.ascontiguousarray(
            Wih0[:, sl, :].reshape(4 * UPC, 2 * E).T)        # [2E, 512]
        whh0T = np.ascontiguousarray(
            Whh0[:, sl, :].reshape(4 * UPC, H).T)            # [H, 512]
        wihrT = np.ascontiguousarray(
            Wih_r[:, :, sl, :].reshape(L - 1, 4 * UPC, H).transpose(0, 2, 1))
        whhrT = np.ascontiguousarray(
            Whh_r[:, :, sl, :].reshape(L - 1, 4 * UPC, H).transpose(0, 2, 1))
        in_maps.append({
            "encs": np.ascontiguousarray(enc[c * BPC:(c + 1) * BPC]),
            "xT": xT,
            "h0T": h0T,
            "c0T": np.ascontiguousarray(c0[:, :, sl].transpose(0, 2, 1)),
            "u": u,
            "wih0T": wih0T,
            "whh0T": whh0T,
            "wihrT": wihrT,
            "whhrT": whhrT,
            "b0": np.ascontiguousarray(b0_full[:, sl].T),            # [128, 4]
            "br": np.ascontiguousarray(
                br_full[:, :, sl].reshape(4 * (L - 1), UPC).T),      # [128, 12]
            "walT": np.ascontiguousarray(Wal[sl, :].T),              # [3072, 128]
            "bal": np.ascontiguousarray(bal[sl])[:, None],           # [128, 1]
            "woutT": np.ascontiguousarray(woutT_pad[:, c * VS:(c + 1) * VS]),
            "bout": np.ascontiguousarray(bout_pad[c * VS:(c + 1) * VS])[None, :],
        })
    return in_maps, key_e


def kernel(**inputs):
    global LAST_EXEC_TIME_NS
    from concourse.bass_utils import run_bass_kernel_spmd

    if "nc" not in _COMPILED:
        _COMPILED["nc"] = _build_nc()
    nc = _COMPILED["nc"]

    in_maps, _ = _host_prep(**inputs)

    trace = os.environ.get("KERNEL_TRACE", "0") == "1"
    res = run_bass_kernel_spmd(nc, in_maps, list(range(NC)), trace=trace)
    LAST_EXEC_TIME_NS = res.exec_time_ns

    r = res.results
    logits = np.concatenate([r[c]["logits_o"] for c in range(NC)], axis=1)[:, :V]
    hn = np.concatenate(
        [r[c]["hnT"].transpose(0, 2, 1) for c in range(NC)], axis=2)
    cn = np.concatenate(
        [r[c]["cnT"].transpose(0, 2, 1) for c in range(NC)], axis=2)
    attns = np.concatenate([r[c]["attnsT"] for c in range(NC)], axis=0)[:, None, :]
    return logits, hn, cn, attns


# revision 3
# speedup vs baseline: 1.0129x; 1.0129x over previous
"""Trainium2 Bass kernel for a single-step 4-layer LSTM decoder with
Bahdanau attention and a 50k-vocab output projection, SPMD across 8
NeuronCores.

Distribution strategy (hardcoded for B=32, S=2048, H=E=1024, V=50257, L=4):
  - Attention is data-parallel over batch: 4 batches per core, streaming
    encoder_outputs once (online unnormalized softmax; the additive-attention
    hidden term v.(Wa_h@out+ba) is constant per row and cancels in softmax,
    and v folds through Wa_e on the host into a single vector u, so the
    whole attention scan is independent of the LSTM).
  - LSTM is tensor-parallel over hidden units: 128 units per core across all
    4 gates, with a small AllGather of h after each layer.
  - Output layer is tensor-parallel over vocab: Wout is pre-transposed,
    zero-padded to 51200 and column-sharded 6400 per core.
  - The embedding table is never shipped: the two gathers (64 rows) happen
    on the host and feed the kernel as a 2048x32 transposed input.

All weight matrices are pre-transposed on the host so every DMA is
contiguous along the partition line.
"""

import os

import numpy as np

B, S, H, E, V, L = 32, 2048, 1024, 1024, 50257, 4
NC = 8
BPC = B // NC          # batches per core (attention)
UPC = H // NC          # hidden units per core (LSTM)
VPAD = 51200           # vocab padded to 8*6400
VS = VPAD // NC        # vocab shard per core
ST = S // 128          # s-tiles per batch (16)
LGC = 512              # logits column chunk

_COMPILED = {}
LAST_EXEC_TIME_NS = None


def _build_nc():
    import concourse.bacc as bacc
    import concourse.mybir as mybir
    import concourse.tile as tile

    f32 = mybir.dt.float32
    AF = mybir.ActivationFunctionType
    OP = mybir.AluOpType

    nc = bacc.Bacc("TRN2", target_bir_lowering=False, debug=False, num_devices=NC)

    # ---- parameters ----
    encs = nc.declare_dram_parameter("encs", [BPC, S, H], f32, isOutput=False)
    xT = nc.declare_dram_parameter("xT", [2 * E, B], f32, isOutput=False)
    h0T = nc.declare_dram_parameter("h0T", [L, H, B], f32, isOutput=False)
    c0T = nc.declare_dram_parameter("c0T", [L, UPC, B], f32, isOutput=False)
    u_p = nc.declare_dram_parameter("u", [1, H], f32, isOutput=False)
    wih0T = nc.declare_dram_parameter("wih0T", [2 * E, 4 * UPC], f32, isOutput=False)
    whh0T = nc.declare_dram_parameter("whh0T", [H, 4 * UPC], f32, isOutput=False)
    wihrT = nc.declare_dram_parameter("wihrT", [L - 1, H, 4 * UPC], f32, isOutput=False)
    whhrT = nc.declare_dram_parameter("whhrT", [L - 1, H, 4 * UPC], f32, isOutput=False)
    b0_p = nc.declare_dram_parameter("b0", [UPC, 4], f32, isOutput=False)
    br_p = nc.declare_dram_parameter("br", [UPC, 4 * (L - 1)], f32, isOutput=False)
    walT = nc.declare_dram_parameter("walT", [2 * H + E, UPC], f32, isOutput=False)
    bal_p = nc.declare_dram_parameter("bal", [UPC, 1], f32, isOutput=False)
    woutT = nc.declare_dram_parameter("woutT", [H, VS], f32, isOutput=False)
    bout_p = nc.declare_dram_parameter("bout", [1, VS], f32, isOutput=False)

    logits_o = nc.declare_dram_parameter("logits_o", [B, VS], f32, isOutput=True)
    hnT_o = nc.declare_dram_parameter("hnT", [L, UPC, B], f32, isOutput=True)
    cnT_o = nc.declare_dram_parameter("cnT", [L, UPC, B], f32, isOutput=True)
    attnsT_o = nc.declare_dram_parameter("attnsT", [BPC, S], f32, isOutput=True)

    grp = [list(range(NC))]

    with tile.TileContext(nc) as tc:
        with (
            tc.tile_pool(name="sb", bufs=1) as sb,
            tc.tile_pool(name="ps", bufs=1, space="PSUM") as ps,
            tc.tile_pool(name="dram", bufs=1, space="DRAM") as dram,
        ):
            # ---------- prelude: small constants ----------
            ones = sb.tile([128, 32], f32, name="ones")
            nc.vector.memset(ones[:], 1.0)
            u_bc = sb.tile([128, H], f32, name="u_bc")
            nc.sync.dma_start(u_bc[:], u_p.ap().broadcast_to([128, H]))
            b0_sb = sb.tile([UPC, 4], f32, name="b0_sb")
            nc.sync.dma_start(b0_sb[:], b0_p[:])
            br_sb = sb.tile([UPC, 4 * (L - 1)], f32, name="br_sb")
            nc.sync.dma_start(br_sb[:], br_p[:])
            bal_sb = sb.tile([UPC, 1], f32, name="bal_sb")
            nc.sync.dma_start(bal_sb[:], bal_p[:])
            c0_sb = []
            for l in range(L):
                c0_l = sb.tile([UPC, B], f32, name=f"c0_{l}")
                nc.sync.dma_start(c0_l[:], c0T[l])
                c0_sb.append(c0_l)
            bout_sb = sb.tile([1, VS], f32, name="bout_sb")
            nc.sync.dma_start(bout_sb[:], bout_p[:])

            # DRAM bounce buffers for collectives
            ag_in = [dram.tile([UPC, B], f32, name=f"ag_in{l}") for l in range(L)]
            ag_out = [dram.tile([H, B], f32, name=f"ag_out{l}") for l in range(L)]
            ctxT_loc = dram.tile([H, BPC], f32, name="ctxT_loc")
            ctx_g = dram.tile([NC * H, BPC], f32, name="ctx_g")
            hid_in = dram.tile([UPC, B], f32, name="hid_in")
            hid_out = dram.tile([H, B], f32, name="hid_out")
            invd_dram = dram.tile([1, BPC], f32, name="invd_dram")

            # ---------- LSTM: 4 layers, hidden-sharded, AllGather per layer ----------
            gp = ps.tile([128, 128], f32, name="gp", bufs=2)

            for l in range(L):
                if l == 0:
                    n_x = (2 * E) // 128  # 16
                    wsrc = [(wih0T, j) for j in range(n_x)] + [
                        (whh0T, j) for j in range(H // 128)
                    ]
                    rsrc = [(xT, j) for j in range(n_x)] + [
                        (h0T, (0, j)) for j in range(H // 128)
                    ]
                else:
                    wsrc = [(wihrT, (l - 1, j)) for j in range(H // 128)] + [
                        (whhrT, (l - 1, j)) for j in range(H // 128)
                    ]
                    rsrc = [(ag_out[l - 1], j) for j in range(H // 128)] + [
                        (h0T, (l, j)) for j in range(H // 128)
                    ]
                kt = len(wsrc)

                w_tiles = []
                for k, (src, idx) in enumerate(wsrc):
                    wt = sb.tile([128, 4 * UPC], f32, name="lw", tag="lw", bufs=26)
                    if isinstance(idx, tuple):
                        li, j = idx
                        nc.sync.dma_start(wt[:], src[li, j * 128:(j + 1) * 128, :])
                    else:
                        nc.sync.dma_start(wt[:], src[idx * 128:(idx + 1) * 128, :])
                    w_tiles.append(wt)
                r_tiles = []
                for k, (src, idx) in enumerate(rsrc):
                    rt = sb.tile([128, B], f32, name="rk", tag="rk", bufs=26)
                    if isinstance(idx, tuple):
                        li, j = idx
                        nc.gpsimd.dma_start(rt[:], src[li, j * 128:(j + 1) * 128, :])
                    else:
                        nc.gpsimd.dma_start(rt[:], src[idx * 128:(idx + 1) * 128, :])
                    r_tiles.append(rt)

                gpt = gp if l == 0 else ps.tile([128, 128], f32, name="gp", tag="gp", bufs=2)
                for gi in range(4):
                    for k in range(kt):
                        nc.tensor.matmul(
                            gpt[:, gi * 32:(gi + 1) * 32],
                            w_tiles[k][:, gi * UPC:(gi + 1) * UPC],
                            r_tiles[k][:],
                            start=(k == 0),
                            stop=(k == kt - 1),
                        )

                bias = b0_sb if l == 0 else br_sb
                boff = 0 if l == 0 else 4 * (l - 1)
                s_i = sb.tile([UPC, B], f32, name="s_i", tag="s_i", bufs=2)
                s_f = sb.tile([UPC, B], f32, name="s_f", tag="s_f", bufs=2)
                tg = sb.tile([UPC, B], f32, name="tg", tag="tg", bufs=2)
                s_o = sb.tile([UPC, B], f32, name="s_o", tag="s_o", bufs=2)
                nc.scalar.activation(s_i[:], gpt[:, 0:32], AF.Sigmoid, bias=bias[:, boff + 0:boff + 1])
                nc.scalar.activation(s_f[:], gpt[:, 32:64], AF.Sigmoid, bias=bias[:, boff + 1:boff + 2])
                nc.scalar.activation(tg[:], gpt[:, 64:96], AF.Tanh, bias=bias[:, boff + 2:boff + 3])
                nc.scalar.activation(s_o[:], gpt[:, 96:128], AF.Sigmoid, bias=bias[:, boff + 3:boff + 4])

                c_new = sb.tile([UPC, B], f32, name="c_new", tag="c_new", bufs=2)
                h_new = sb.tile([UPC, B], f32, name="h_new", tag="h_new", bufs=2)
                tcn = sb.tile([UPC, B], f32, name="tcn", tag="tcn", bufs=2)
                m2 = sb.tile([UPC, B], f32, name="m2", tag="m2", bufs=2)
                nc.vector.tensor_mul(c_new[:], s_f[:], c0_sb[l][:])
                nc.vector.tensor_mul(m2[:], s_i[:], tg[:])
                nc.vector.tensor_add(c_new[:], c_new[:], m2[:])
                nc.scalar.activation(tcn[:], c_new[:], AF.Tanh)
                nc.vector.tensor_mul(h_new[:], s_o[:], tcn[:])

                nc.gpsimd.dma_start(cnT_o[l], c_new[:])
                nc.gpsimd.dma_start(hnT_o[l], h_new[:])
                nc.gpsimd.dma_start(ag_in[l][:], h_new[:])
                nc.gpsimd.collective_compute(
                    "AllGather", OP.bypass, replica_groups=grp,
                    ins=[ag_in[l].opt()], outs=[ag_out[l].opt()],
                )

            # ---------- attention: 4 batches/core, stream enc once ----------
            e_all, p_all, acc_b = [], [], []
            for b in range(BPC):
                e_all.append(sb.tile([128, ST], f32, name=f"e_all{b}"))
                p_all.append(sb.tile([128, ST], f32, name=f"p_all{b}"))
                acc_b.append(sb.tile([128, H], f32, name=f"acc{b}"))
            scratch = sb.tile([128, H], f32, name="scratch")
            d_ps = ps.tile([1, BPC], f32, name="d_ps")
            ctxf = ps.tile([128, 8], f32, name="ctxf", bufs=2)

            for b in range(BPC):
                for t in range(ST):
                    et = sb.tile([128, H], f32, name="enc_t", tag="enc_t", bufs=6)
                    nc.sync.dma_start(et[:], encs[b, t * 128:(t + 1) * 128, :])
                    nc.vector.scalar_tensor_tensor(
                        out=scratch[:], in0=et[:], scalar=1.0, in1=u_bc[:],
                        op0=OP.mult, op1=OP.mult, accum_out=e_all[b][:, t:t + 1],
                    )
                    nc.scalar.activation(
                        p_all[b][:, t:t + 1], e_all[b][:, t:t + 1], AF.Exp
                    )
                    if t == 0:
                        nc.vector.tensor_scalar_mul(
                            acc_b[b][:], et[:], p_all[b][:, 0:1]
                        )
                    else:
                        nc.vector.scalar_tensor_tensor(
                            out=acc_b[b][:], in0=et[:], scalar=p_all[b][:, t:t + 1],
                            in1=acc_b[b][:], op0=OP.mult, op1=OP.add,
                        )
                d_col = sb.tile([128, 1], f32, name="d_col", tag="d_col", bufs=2)
                nc.vector.reduce_sum(d_col[:], p_all[b][:], axis=mybir.AxisListType.X)
                nc.tensor.matmul(
                    d_ps[0:1, b:b + 1], d_col[:], ones[:, 0:1], start=True, stop=True
                )

            d_sb = sb.tile([1, BPC], f32, name="d_sb")
            inv_d = sb.tile([1, BPC], f32, name="inv_d")
            nc.vector.tensor_copy(d_sb[:], d_ps[:])
            nc.vector.reciprocal(inv_d[:], d_sb[:])
            nc.gpsimd.dma_start(invd_dram[:], inv_d[:])
            invd_bc = sb.tile([128, BPC], f32, name="invd_bc")
            nc.gpsimd.dma_start(invd_bc[:], invd_dram.broadcast_to([128, BPC]))

            for b in range(BPC):
                at_sb = sb.tile([128, ST], f32, name="at_sb", tag="at_sb", bufs=2)
                nc.vector.tensor_scalar_mul(at_sb[:], p_all[b][:], invd_bc[:, b:b + 1])
                nc.gpsimd.dma_start(
                    attnsT_o[b].rearrange("(t p) -> p t", p=128), at_sb[:]
                )
                ctxft = ctxf if b == 0 else ps.tile([128, 8], f32, name="ctxf", tag="ctxf", bufs=2)
                for m in range(8):
                    nc.tensor.matmul(
                        ctxft[:, m:m + 1],
                        acc_b[b][:, m * 128:(m + 1) * 128],
                        ones[:, 0:1],
                        start=True, stop=True,
                    )
                cx_sb = sb.tile([128, 8], f32, name="cx_sb", tag="cx_sb", bufs=2)
                nc.vector.tensor_scalar_mul(cx_sb[:], ctxft[:], invd_bc[:, b:b + 1])
                nc.gpsimd.dma_start(
                    ctxT_loc[:, b].rearrange("(m p) -> p m", p=128), cx_sb[:]
                )

            nc.gpsimd.collective_compute(
                "AllGather", OP.bypass, replica_groups=grp,
                ins=[ctxT_loc.opt()], outs=[ctx_g.opt()],
            )

            # ---------- hidden = tanh(Wal @ [out; context; key_e] + bal), sharded ----------
            wal_tiles = []
            KT = (2 * H + E) // 128  # 24
            for k in range(KT):
                wt = sb.tile([128, UPC], f32, name="walt", tag="walt", bufs=KT)
                nc.sync.dma_start(wt[:], walT[k * 128:(k + 1) * 128, :])
                wal_tiles.append(wt)
            hr_tiles = []
            ctx_g3 = ctx_g.rearrange("(c h) b -> h c b", c=NC)
            for k in range(KT):
                rt = sb.tile([128, B], f32, name="hrk", tag="rk", bufs=26)
                if k < 8:
                    nc.gpsimd.dma_start(rt[:], ag_out[L - 1][k * 128:(k + 1) * 128, :])
                elif k < 16:
                    j = k - 8
                    nc.gpsimd.dma_start(
                        rt.rearrange("p (c b) -> p c b", c=NC),
                        ctx_g3[j * 128:(j + 1) * 128],
                    )
                else:
                    j = k - 16
                    nc.gpsimd.dma_start(rt[:], xT[E + j * 128:E + (j + 1) * 128, :])
                hr_tiles.append(rt)

            hid_ps = ps.tile([UPC, B], f32, name="hid_ps")
            for k in range(KT):
                nc.tensor.matmul(
                    hid_ps[:], wal_tiles[k][:], hr_tiles[k][:],
                    start=(k == 0), stop=(k == KT - 1),
                )
            hidT_sb = sb.tile([UPC, B], f32, name="hidT_sb")
            nc.scalar.activation(hidT_sb[:], hid_ps[:], AF.Tanh, bias=bal_sb[:, 0:1])
            nc.gpsimd.dma_start(hid_in[:], hidT_sb[:])
            nc.gpsimd.collective_compute(
                "AllGather", OP.bypass, replica_groups=grp,
                ins=[hid_in.opt()], outs=[hid_out.opt()],
            )
            hT_sb = sb.tile([128, 8 * B], f32, name="hT_sb")
            for k in range(8):
                nc.gpsimd.dma_start(
                    hT_sb[:, k * B:(k + 1) * B], hid_out[k * 128:(k + 1) * 128, :]
                )

            # ---------- logits: vocab-sharded output projection ----------
            n_ch = (VS + LGC - 1) // LGC
            for ch in range(n_ch):
                c0_ = ch * LGC
                cw = min(LGC, VS - c0_)
                lg = ps.tile([B, LGC], f32, name="lg", tag="lg", bufs=2)
                for k in range(8):
                    wt = sb.tile([128, LGC], f32, name="wo", tag="wo", bufs=4)
                    nc.sync.dma_start(
                        wt[:, 0:cw], woutT[k * 128:(k + 1) * 128, c0_:c0_ + cw]
                    )
                    nc.tensor.matmul(
                        lg[:, 0:cw], hT_sb[:, k * B:(k + 1) * B], wt[:, 0:cw],
                        start=(k == 0), stop=False,
                    )
                nc.tensor.matmul(
                    lg[:, 0:cw], ones[0:1, :], bout_sb[0:1, c0_:c0_ + cw],
                    start=False, stop=True,
                )
                lsb = sb.tile([B, LGC], f32, name="lsb", tag="lsb", bufs=3)
                nc.vector.tensor_copy(lsb[:, 0:cw], lg[:, 0:cw])
                nc.sync.dma_start(logits_o[:, c0_:c0_ + cw], lsb[:, 0:cw])

    nc.finalize()
    return nc


def _host_prep(encoder_outputs, input_seq, keyword, h0, c0, emb,
               Wih0, Whh0, bih0, bhh0, Wih_r, Whh_r, bih_r, bhh_r,
               Wa, ba, v, Wal, bal, Wout, bout):
    f = np.float32
    enc = np.asarray(encoder_outputs, f)
    emb = np.asarray(emb, f)
    idx_sos = np.asarray(input_seq).astype(np.int64)[:, 0]
    idx_key = np.asarray(keyword).astype(np.int64)
    sos = emb[idx_sos]                      # [B, E]
    key_e = emb[idx_key]                    # [B, E]
    xT = np.ascontiguousarray(np.concatenate([sos, key_e], axis=1).T)  # [2E, B]

    Wa = np.asarray(Wa, f)
    v_ = np.asarray(v, f)
    u = (Wa[:, H:].T @ v_)[None, :].astype(f)  # [1, H]

    h0 = np.asarray(h0, f)
    c0 = np.asarray(c0, f)
    h0T = np.ascontiguousarray(h0.transpose(0, 2, 1))  # [L, H, B]

    Wih0 = np.asarray(Wih0, f).reshape(4, H, 2 * E)
    Whh0 = np.asarray(Whh0, f).reshape(4, H, H)
    Wih_r = np.asarray(Wih_r, f).reshape(L - 1, 4, H, H)
    Whh_r = np.asarray(Whh_r, f).reshape(L - 1, 4, H, H)
    b0_full = (np.asarray(bih0, f) + np.asarray(bhh0, f)).reshape(4, H)
    br_full = (np.asarray(bih_r, f) + np.asarray(bhh_r, f)).reshape(L - 1, 4, H)

    Wal = np.asarray(Wal, f)
    bal = np.asarray(bal, f)
    Wout = np.asarray(Wout, f)
    bout = np.asarray(bout, f)

    woutT_pad = np.zeros((H, VPAD), f)
    woutT_pad[:, :V] = Wout.T
    bout_pad = np.zeros(VPAD, f)
    bout_pad[:V] = bout

    in_maps = []
    for c in range(NC):
        sl = slice(c * UPC, (c + 1) * UPC)
        wih0T = np.ascontiguousarray(
            Wih0[:, sl, :].reshape(4 * UPC, 2 * E).T)        # [2E, 512]
        whh0T = np.ascontiguousarray(
            Whh0[:, sl, :].reshape(4 * UPC, H).T)            # [H, 512]
        wihrT = np.ascontiguousarray(
            Wih_r[:, :, sl, :].reshape(L - 1, 4 * UPC, H).transpose(0, 2, 1))
        whhrT = np.ascontiguousarray(
            Whh_r[:, :, sl, :].reshape(L - 1, 4 * UPC, H).transpose(0, 2, 1))
        in_maps.append({
            "encs": np.ascontiguousarray(enc[c * BPC:(c + 1) * BPC]),
            "xT": xT,
            "h0T": h0T,
            "c0T": np.ascontiguousarray(c0[:, :, sl].transpose(0, 2, 1)),
            "u": u,
            "wih0T": wih0T,
            "whh0T": whh0T,
            "wihrT": wihrT,
            "whhrT": whhrT,
            "b0": np.ascontiguousarray(b0_full[:, sl].T),            # [128, 4]
            "br": np.ascontiguousarray(
                br_full[:, :, sl].reshape(4 * (L - 1), UPC).T),      # [128, 12]
            "walT": np.ascontiguousarray(Wal[sl, :].T),              # [3072, 128]
            "bal": np.ascontiguousarray(bal[sl])[:, None],           # [128, 1]
            "woutT": np.ascontiguousarray(woutT_pad[:, c * VS:(c + 1) * VS]),
            "bout": np.ascontiguousarray(bout_pad[c * VS:(c + 1) * VS])[None, :],
        })
    return in_maps, key_e


def kernel(**inputs):
    global LAST_EXEC_TIME_NS
    from concourse.bass_utils import run_bass_kernel_spmd

    if "nc" not in _COMPILED:
        _COMPILED["nc"] = _build_nc()
    nc = _COMPILED["nc"]

    in_maps, _ = _host_prep(**inputs)

    trace = os.environ.get("KERNEL_TRACE", "0") == "1"
    res = run_bass_kernel_spmd(nc, in_maps, list(range(NC)), trace=trace)
    LAST_EXEC_TIME_NS = res.exec_time_ns
    _COMPILED["last_res"] = res

    r = res.results
    logits = np.concatenate([r[c]["logits_o"] for c in range(NC)], axis=1)[:, :V]
    hn = np.concatenate(
        [r[c]["hnT"].transpose(0, 2, 1) for c in range(NC)], axis=2)
    cn = np.concatenate(
        [r[c]["cnT"].transpose(0, 2, 1) for c in range(NC)], axis=2)
    attns = np.concatenate([r[c]["attnsT"] for c in range(NC)], axis=0)[:, None, :]
    return logits, hn, cn, attns
